# revision 1
# baseline (speedup 1.0000x reference)
"""DenseCaps routing kernel for 8x Trainium2 NeuronCores.

Shapes (hardcoded): inputs (16, 2048, 16) f32, w (2048, 16, 64, 32) f32.
Sharding: ch_i (2048) split 8 ways -> 256 i's per core. Each core computes
u[b, i_loc, j, m] via a block-diagonal stationary matmul streaming its w
shard once, keeps u resident in SBUF (bf16), runs the 3-iteration dynamic
routing locally, and AllReduces the tiny s[b, j, m] partial sums.
Output v (16, 64, 32) f32 is identical on all cores; core 0's is returned.
"""

import sys
from contextlib import ExitStack

import numpy as np

sys.path.insert(0, "/opt/trn_rl_repo")

import concourse.bass as bass
import concourse.bacc as bacc
import concourse.tile as tile
from concourse import mybir
from concourse.bass_utils import run_bass_kernel_spmd

F32 = mybir.dt.float32
BF16 = mybir.dt.bfloat16

B = 16
CH_I = 2048
N_I = 16
CH_J = 64
N_J = 32
JM = CH_J * N_J  # 2048
N_CORES = 8
I_LOC = CH_I // N_CORES  # 256
N_T = I_LOC // 8  # 32 production tiles, 8 i's each
EPS = 1e-7

_CACHE = {}


def _build_program(trace=False):
    nc = bacc.Bacc("TRN2", target_bir_lowering=False, debug=False,
                   num_devices=N_CORES)

    xblk_d = nc.dram_tensor("xblk", [N_T, 128, 128], F32, kind="ExternalInput")
    w_d = nc.dram_tensor("w", [I_LOC * N_I, JM], F32, kind="ExternalInput")
    s16_d = nc.dram_tensor("s16", [128, 16], BF16, kind="ExternalInput")
    r8_d = nc.dram_tensor("r8", [16, 128], F32, kind="ExternalInput")
    v_d = nc.dram_tensor("v", [B, JM], F32, kind="ExternalOutput")

    cc_in = [nc.dram_tensor(f"cc_in{r}", [B, JM], F32) for r in range(3)]
    cc_out = [nc.dram_tensor(f"cc_out{r}", [B, JM], F32, addr_space="Shared")
              for r in range(3)]

    with tile.TileContext(nc) as tc, ExitStack() as ctx:
        _kernel_body(ctx, tc, xblk_d, w_d, s16_d, r8_d, v_d, cc_in, cc_out)
    nc.compile()
    return nc


def _kernel_body(ctx, tc, xblk_d, w_d, s16_d, r8_d, v_d, cc_in, cc_out):
    nc = tc.nc
    Act = mybir.ActivationFunctionType
    Alu = mybir.AluOpType
    groups = [list(range(N_CORES))]

    const_pool = ctx.enter_context(tc.tile_pool(name="consts", bufs=1))
    s16 = const_pool.tile([128, 16], BF16)
    nc.sync.dma_start(s16[:], s16_d[:])
    r8 = const_pool.tile([16, 128], F32)
    nc.sync.dma_start(r8[:], r8_d[:])
    eps_t = const_pool.tile([16, 1], F32)
    nc.vector.memset(eps_t[:], EPS)

    u_pool = ctx.enter_context(tc.tile_pool(name="u", bufs=N_T))
    u_tiles = []

    # small persistent buffers
    sv_pool = ctx.enter_context(tc.tile_pool(name="sv", bufs=1))
    a1_pool = ctx.enter_context(tc.tile_pool(name="a1", bufs=N_T))
    a1_tiles = []

    # ---------------- Phase A: produce u, accumulate s0 ----------------
    with tc.tile_pool(name="wbuf", bufs=2) as w_pool, \
         tc.tile_pool(name="xbuf", bufs=2) as x_pool, \
         tc.tile_pool(name="uprod", bufs=2, space="PSUM") as up_pool, \
         tc.tile_pool(name="s0ps", bufs=1, space="PSUM") as s0_pool:
        s0_ps = s0_pool.tile([16, JM], F32)
        for t in range(N_T):
            wt = w_pool.tile([128, JM], F32)
            nc.sync.dma_start(wt[:], w_d[t * 128:(t + 1) * 128, :])
            xt = x_pool.tile([128, 128], F32)
            nc.sync.dma_start(xt[:], xblk_d[t])
            ut = u_pool.tile([128, JM], BF16)
            u_tiles.append(ut)
            for h in range(2):
                up = up_pool.tile([128, 1024], F32)
                for q in range(2):
                    nc.tensor.matmul(
                        up[:, q * 512:(q + 1) * 512], xt[:],
                        wt[:, h * 1024 + q * 512: h * 1024 + (q + 1) * 512],
                        start=True, stop=True)
                dst = ut[:, h * 1024:(h + 1) * 1024]
                if h == 0:
                    nc.scalar.activation(dst, up[:], Act.Copy)
                else:
                    nc.vector.tensor_copy(dst, up[:])
                for q in range(2):
                    o0 = h * 1024 + q * 512
                    nc.tensor.matmul(
                        s0_ps[:, o0:o0 + 512], s16[:],
                        ut[:, o0:o0 + 512],
                        start=(t == 0), stop=(t == N_T - 1))
        s_sb = sv_pool.tile([16, JM], F32, tag="s_sb")
        nc.scalar.activation(s_sb[:, :1024], s0_ps[:, :1024], Act.Copy)
        nc.vector.tensor_copy(s_sb[:, 1024:], s0_ps[:, 1024:])

    def allreduce_squash(r, s_sb):
        """AllReduce partial s, then squash -> v_sb (f32 (16, JM))."""
        nc.sync.dma_start(cc_in[r][:], s_sb[:])
        nc.gpsimd.collective_compute(
            "AllReduce", Alu.add, replica_groups=groups,
            ins=[cc_in[r][:]], outs=[cc_out[r][:]])
        sf = sv_pool.tile([16, JM], F32, tag="sf")
        nc.sync.dma_start(sf[:], cc_out[r][:])
        t2 = sv_pool.tile([16, JM], F32, tag="t2")
        nc.vector.tensor_mul(t2[:], sf[:], sf[:])
        sq = sv_pool.tile([16, CH_J], F32, tag="sq")
        nc.vector.tensor_reduce(
            sq[:], t2[:].rearrange("p (j m) -> p j m", m=N_J),
            axis=mybir.AxisListType.X, op=Alu.add)
        rt = sv_pool.tile([16, CH_J], F32, tag="rt")
        nc.scalar.activation(rt[:], sq[:], Act.Sqrt, bias=eps_t[:])
        onep = sv_pool.tile([16, CH_J], F32, tag="onep")
        nc.vector.tensor_scalar_add(onep[:], sq[:], 1.0)
        den = sv_pool.tile([16, CH_J], F32, tag="den")
        nc.vector.tensor_mul(den[:], rt[:], onep[:])
        rec = sv_pool.tile([16, CH_J], F32, tag="rec")
        nc.vector.reciprocal(rec[:], den[:])
        fs = sv_pool.tile([16, CH_J], F32, tag="fs")
        nc.vector.tensor_mul(fs[:], sq[:], rec[:])
        vv = sv_pool.tile([16, JM], F32, tag="vv")
        nc.vector.tensor_mul(
            vv[:].rearrange("p (j m) -> p j m", m=N_J),
            sf[:].rearrange("p (j m) -> p j m", m=N_J),
            fs[:].unsqueeze(-1).broadcast_to((16, CH_J, N_J)))
        return vv

    v_sb = allreduce_squash(0, s_sb)

    # ---------------- Phases C: routing passes r=1,2 ----------------
    for r in (1, 2):
        with tc.tile_pool(name=f"vr{r}", bufs=1) as vr_pool, \
             tc.tile_pool(name=f"scr{r}", bufs=2) as scr_pool, \
             tc.tile_pool(name=f"sm{r}", bufs=4) as sm_pool, \
             tc.tile_pool(name=f"vrps{r}", bufs=2, space="PSUM") as vr_ps_pool, \
             tc.tile_pool(name=f"sps{r}", bufs=1, space="PSUM") as s_ps_pool:
            # v_rep (128, JM) bf16: broadcast v over the 8 i-slots
            v_rep = vr_pool.tile([128, JM], BF16)
            for q in range(4):
                vp = vr_ps_pool.tile([128, 512], F32)
                nc.tensor.matmul(vp[:], r8[:],
                                 v_sb[:, q * 512:(q + 1) * 512],
                                 start=True, stop=True)
                if q % 2 == 0:
                    nc.scalar.activation(v_rep[:, q * 512:(q + 1) * 512],
                                         vp[:], Act.Copy)
                else:
                    nc.vector.tensor_copy(v_rep[:, q * 512:(q + 1) * 512],
                                          vp[:])
            s_ps = s_ps_pool.tile([16, JM], F32)
            for t in range(N_T):
                ut = u_tiles[t]
                uv = scr_pool.tile([128, JM], BF16, tag="uv")
                nc.vector.tensor_mul(uv[:], ut[:], v_rep[:])
                if r == 1:
                    a1 = a1_pool.tile([128, CH_J], F32)
                    a1_tiles.append(a1)
                    nc.vector.tensor_reduce(
                        a1[:], uv[:].rearrange("p (j m) -> p j m", m=N_J),
                        axis=mybir.AxisListType.X, op=Alu.add)
                    logits = a1
                else:
                    a2 = sm_pool.tile([128, CH_J], F32, tag="a2")
                    nc.vector.tensor_reduce(
                        a2[:], uv[:].rearrange("p (j m) -> p j m", m=N_J),
                        axis=mybir.AxisListType.X, op=Alu.add)
                    lg = sm_pool.tile([128, CH_J], F32, tag="lg")
                    nc.vector.tensor_add(lg[:], a2[:], a1_tiles[t][:])
                    logits = lg
                expt = sm_pool.tile([128, CH_J], F32, tag="expt")
                se = sm_pool.tile([128, 1], F32, tag="se")
                nc.scalar.activation(expt[:], logits[:], Act.Exp,
                                     scale=8.0, accum_out=se[:])
                se2 = sm_pool.tile([128, 1], F32, tag="se2")
                nc.vector.tensor_scalar_mul(se2[:], se[:], 1.0 / CH_J)
                rs = sm_pool.tile([128, 1], F32, tag="rs")
                nc.vector.reciprocal(rs[:], se2[:])
                ct = sm_pool.tile([128, CH_J], BF16, tag="ct")
                nc.vector.tensor_scalar_mul(ct[:], expt[:], rs[:])
                cexp = scr_pool.tile([128, JM], BF16, tag="cexp")
                nc.scalar.activation(
                    cexp[:].rearrange("p (j m) -> p j m", m=N_J),
                    ct[:].unsqueeze(-1).broadcast_to((128, CH_J, N_J)),
                    Act.Copy)
                cu = scr_pool.tile([128, JM], BF16, tag="cu")
                nc.vector.tensor_mul(cu[:], ut[:], cexp[:])
                for q in range(4):
                    nc.tensor.matmul(
                        s_ps[:, q * 512:(q + 1) * 512], s16[:],
                        cu[:, q * 512:(q + 1) * 512],
                        start=(t == 0), stop=(t == N_T - 1))
            s_sb2 = sv_pool.tile([16, JM], F32, tag="s_sb")
            nc.scalar.activation(s_sb2[:, :1024], s_ps[:, :1024], Act.Copy)
            nc.vector.tensor_copy(s_sb2[:, 1024:], s_ps[:, 1024:])
        v_sb = allreduce_squash(r, s_sb2)

    nc.sync.dma_start(v_d[:], v_sb[:])


def _host_inputs(inputs, w):
    """Build per-core input maps (host-side shard + block-diag pack)."""
    x = np.asarray(inputs, dtype=np.float32)
    w = np.asarray(w, dtype=np.float32)
    s16 = np.zeros((128, 16), dtype=np.float32)
    for i8 in range(8):
        for b in range(16):
            s16[i8 * 16 + b, b] = 1.0
    import ml_dtypes
    s16 = s16.astype(ml_dtypes.bfloat16)
    r8 = np.zeros((16, 128), dtype=np.float32)
    for i8 in range(8):
        for b in range(16):
            r8[b, i8 * 16 + b] = 1.0
    in_maps = []
    for k in range(N_CORES):
        i0 = k * I_LOC
        wk = w[i0:i0 + I_LOC].reshape(I_LOC * N_I, JM).copy()
        xk = x[:, i0:i0 + I_LOC, :]  # (B, 256, 16)
        xblk = np.zeros((N_T, 128, 128), dtype=np.float32)
        # xblk[t, i8*16+n, i8*16+b] = x[b, i0+8t+i8, n]
        xv = xk.transpose(1, 2, 0).reshape(N_T, 8, N_I, B)  # (t, i8, n, b)
        for i8 in range(8):
            xblk[:, i8 * 16:i8 * 16 + N_I, i8 * 16:i8 * 16 + B] = xv[:, i8]
        in_maps.append({"xblk": xblk, "w": wk, "s16": s16, "r8": r8})
    return in_maps


def kernel(inputs, w, _trace=False):
    key = "nc"
    if key not in _CACHE:
        _CACHE[key] = _build_program()
    nc = _CACHE[key]
    in_maps = _host_inputs(inputs, w)
    res = run_bass_kernel_spmd(nc, in_maps, list(range(N_CORES)),
                               trace=_trace)
    v = res.results[0]["v"].reshape(B, CH_J, N_J).astype(np.float32)
    if _trace:
        kernel._last = res
    return v



# revision 4
# speedup vs baseline: 1.3141x; 1.3141x over previous
"""DenseCaps routing kernel for 8x Trainium2 NeuronCores.

Shapes (hardcoded): inputs (16, 2048, 16) f32, w (2048, 16, 64, 32) f32.
Sharding: ch_i (2048) split 8 ways -> 256 i's per core. Each core computes
u[b, i_loc, j, m] via a block-diagonal stationary matmul streaming its w
shard once (bf16), keeps u resident in SBUF (bf16), runs the 3-iteration
dynamic routing locally, and AllReduces the small s[b, j, m] partial sums.

Free-dim layout is "paired": f = jp*64 + m*2 + j2 with j = 2*jp + j2.
This makes the c-weighting of u a single 2x-mode tensor_tensor with a
stride-0 broadcast AP (no materialized broadcast), and the m-reduction a
tree of 2x-mode adds instead of a 1x tensor_reduce.

Output v (16, 64, 32) f32 is identical on all cores; core 0's is returned.
"""

import sys
from contextlib import ExitStack

import numpy as np

sys.path.insert(0, "/opt/trn_rl_repo")

import concourse.bass as bass
import concourse.bacc as bacc
import concourse.tile as tile
from concourse import mybir
from concourse.bass_utils import run_bass_kernel_spmd

F32 = mybir.dt.float32
BF16 = mybir.dt.bfloat16

B = 16
CH_I = 2048
N_I = 16
CH_J = 64
N_J = 32
JM = CH_J * N_J  # 2048
JP = CH_J // 2  # 32 j-pairs
N_CORES = 8
I_LOC = CH_I // N_CORES  # 256
N_T = I_LOC // 8  # 32 production tiles, 8 i's each
EPS = 1e-7

_CACHE = {}


def _build_program(trace=False):
    nc = bacc.Bacc("TRN2", target_bir_lowering=False, debug=False,
                   num_devices=N_CORES)

    xblk_d = nc.dram_tensor("xblk", [N_T, 128, 128], BF16, kind="ExternalInput")
    w_d = nc.dram_tensor("w", [I_LOC * N_I, JM], BF16, kind="ExternalInput")
    s16_d = nc.dram_tensor("s16", [128, 16], BF16, kind="ExternalInput")
    r8_d = nc.dram_tensor("r8", [16, 128], BF16, kind="ExternalInput")
    v_d = nc.dram_tensor("v", [B, JM], F32, kind="ExternalOutput")

    cc_in = [nc.dram_tensor(f"cc_in{r}", [B, JM], F32) for r in range(3)]
    cc_out = [nc.dram_tensor(f"cc_out{r}", [B, JM], F32, addr_space="Shared")
              for r in range(3)]

    with tile.TileContext(nc) as tc, ExitStack() as ctx:
        _kernel_body(ctx, tc, xblk_d, w_d, s16_d, r8_d, v_d, cc_in, cc_out)
    nc.compile()
    return nc


def _pv(ap):
    """View a [P, JM] AP as [P, jp, m, j2]."""
    return ap.rearrange("p (jp m j2) -> p jp m j2", m=N_J, j2=2)


def _kernel_body(ctx, tc, xblk_d, w_d, s16_d, r8_d, v_d, cc_in, cc_out):
    nc = tc.nc
    Act = mybir.ActivationFunctionType
    Alu = mybir.AluOpType
    groups = [list(range(N_CORES))]

    const_pool = ctx.enter_context(tc.tile_pool(name="consts", bufs=1))
    s16 = const_pool.tile([128, 16], BF16)
    nc.sync.dma_start(s16[:], s16_d[:])
    r8 = const_pool.tile([16, 128], BF16)
    nc.sync.dma_start(r8[:], r8_d[:])
    eps_t = const_pool.tile([16, 1], F32)
    nc.vector.memset(eps_t[:], EPS)

    u_pool = ctx.enter_context(tc.tile_pool(name="u", bufs=N_T))
    u_tiles = []

    # small persistent buffers
    sv_pool = ctx.enter_context(tc.tile_pool(name="sv", bufs=1))
    a1_pool = ctx.enter_context(tc.tile_pool(name="a1", bufs=N_T))
    a1_tiles = []

    # ---------------- Phase A: produce u, accumulate s0 ----------------
    # Software-pipelined: s0 matmuls for tile t are issued during tile t+1
    # so the PE queue never stalls on the PSUM->SBUF copy of tile t.
    with tc.tile_pool(name="wbuf", bufs=3) as w_pool, \
         tc.tile_pool(name="xbuf", bufs=3) as x_pool, \
         tc.tile_pool(name="uprod", bufs=2, space="PSUM") as up_pool, \
         tc.tile_pool(name="s0ps", bufs=1, space="PSUM") as s0_pool:
        s0_ps = s0_pool.tile([16, JM], F32)

        def issue_s0(t):
            ut = u_tiles[t]
            for q in range(4):
                o0 = q * 512
                nc.tensor.matmul(
                    s0_ps[:, o0:o0 + 512], s16[:], ut[:, o0:o0 + 512],
                    start=(t == 0), stop=(t == N_T - 1))

        for t in range(N_T):
            wt = w_pool.tile([128, JM], BF16)
            nc.sync.dma_start(wt[:], w_d[t * 128:(t + 1) * 128, :])
            xt = x_pool.tile([128, 128], BF16)
            nc.sync.dma_start(xt[:], xblk_d[t])
            ut = u_pool.tile([128, JM], BF16)
            u_tiles.append(ut)
            ups = []
            for h in range(2):
                up = up_pool.tile([128, 1024], F32)
                ups.append(up)
                for q in range(2):
                    nc.tensor.matmul(
                        up[:, q * 512:(q + 1) * 512], xt[:],
                        wt[:, h * 1024 + q * 512: h * 1024 + (q + 1) * 512],
                        start=True, stop=True)
            # copies split across Scalar and Vector engines
            nc.scalar.activation(ut[:, :1024], ups[0][:], Act.Copy)
            nc.vector.tensor_copy(ut[:, 1024:], ups[1][:])
            if t > 0:
                issue_s0(t - 1)
        issue_s0(N_T - 1)
        s_sb = sv_pool.tile([16, JM], F32, tag="s_sb")
        nc.scalar.activation(s_sb[:, :1024], s0_ps[:, :1024], Act.Copy)
        nc.vector.tensor_copy(s_sb[:, 1024:], s0_ps[:, 1024:])

    def allreduce_squash(r, s_sb):
        """AllReduce partial s, then squash -> v (bf16 [16, JM], and f32
        final for r=2)."""
        nc.sync.dma_start(cc_in[r][:], s_sb[:])
        nc.gpsimd.collective_compute(
            "AllReduce", Alu.add, replica_groups=groups,
            ins=[cc_in[r][:]], outs=[cc_out[r][:]])
        sf = sv_pool.tile([16, JM], F32, tag="sf")
        nc.sync.dma_start(sf[:], cc_out[r][:])
        # t2 = sf*sf, halves on Scalar (Square) and Vector
        t2 = sv_pool.tile([16, JM], BF16, tag="t2")
        nc.scalar.activation(t2[:, :1024], sf[:, :1024], Act.Square)
        nc.vector.tensor_mul(t2[:, 1024:], sf[:, 1024:], sf[:, 1024:])
        # tree-reduce over m (outer-of-inner): [16,(jp,m,j2)] -> [16,(jp,j2)]
        t2v = _pv(t2[:])
        q1 = sv_pool.tile([16, JP * 16 * 2], BF16, tag="q1")
        nc.vector.tensor_add(
            q1[:].rearrange("p (jp m j2) -> p jp m j2", m=16, j2=2),
            t2v[:, :, 0:16, :], t2v[:, :, 16:32, :])
        q1v = q1[:].rearrange("p (jp m j2) -> p jp m j2", m=16, j2=2)
        q2 = sv_pool.tile([16, JP * 8 * 2], BF16, tag="q2")
        nc.vector.tensor_add(
            q2[:].rearrange("p (jp m j2) -> p jp m j2", m=8, j2=2),
            q1v[:, :, 0:8, :], q1v[:, :, 8:16, :])
        q2v = q2[:].rearrange("p (jp m j2) -> p jp m j2", m=8, j2=2)
        q3 = sv_pool.tile([16, JP * 4 * 2], F32, tag="q3")
        nc.vector.tensor_add(
            q3[:].rearrange("p (jp m j2) -> p jp m j2", m=4, j2=2),
            q2v[:, :, 0:4, :], q2v[:, :, 4:8, :])
        q3v = q3[:].rearrange("p (jp m j2) -> p jp m j2", m=4, j2=2)
        q4 = sv_pool.tile([16, JP * 2 * 2], F32, tag="q4")
        nc.vector.tensor_add(
            q4[:].rearrange("p (jp m j2) -> p jp m j2", m=2, j2=2),
            q3v[:, :, 0:2, :], q3v[:, :, 2:4, :])
        q4v = q4[:].rearrange("p (jp m j2) -> p jp m j2", m=2, j2=2)
        sq = sv_pool.tile([16, CH_J], F32, tag="sq")
        nc.vector.tensor_add(
            sq[:].rearrange("p (jp m j2) -> p jp m j2", m=1, j2=2),
            q4v[:, :, 0:1, :], q4v[:, :, 1:2, :])
        # squash scalars
        rt = sv_pool.tile([16, CH_J], F32, tag="rt")
        nc.scalar.activation(rt[:], sq[:], Act.Sqrt, bias=eps_t[:])
        onep = sv_pool.tile([16, CH_J], F32, tag="onep")
        nc.vector.tensor_scalar_add(onep[:], sq[:], 1.0)
        den = sv_pool.tile([16, CH_J], F32, tag="den")
        nc.vector.tensor_mul(den[:], rt[:], onep[:])
        rec = sv_pool.tile([16, CH_J], F32, tag="rec")
        nc.vector.reciprocal(rec[:], den[:])
        fs = sv_pool.tile([16, CH_J], F32, tag="fs")
        nc.vector.tensor_mul(fs[:], sq[:], rec[:])
        # v = sf * fs  (fs broadcast over m via stride-0 AP)
        fsb = fs[:].rearrange("p (jp j2) -> p jp j2", j2=2) \
            .unsqueeze(2).broadcast_to((16, JP, N_J, 2))
        vv = sv_pool.tile([16, JM], F32 if r == 2 else BF16, tag="vv")
        nc.vector.tensor_mul(_pv(vv[:]), _pv(sf[:]), fsb)
        return vv

    v_sb = allreduce_squash(0, s_sb)

    # ---------------- Phases C: routing passes r=1,2 ----------------
    for r in (1, 2):
        with tc.tile_pool(name=f"vr{r}", bufs=1) as vr_pool, \
             tc.tile_pool(name=f"scr{r}", bufs=2) as scr_pool, \
             tc.tile_pool(name=f"tr{r}", bufs=2) as tr_pool, \
             tc.tile_pool(name=f"sm{r}", bufs=3) as sm_pool, \
             tc.tile_pool(name=f"vrps{r}", bufs=2, space="PSUM") as vr_ps_pool, \
             tc.tile_pool(name=f"sps{r}", bufs=1, space="PSUM") as s_ps_pool:
            # v_rep (128, JM) bf16: broadcast v over the 8 i-slots
            v_rep = vr_pool.tile([128, JM], BF16)
            for h in range(2):
                vp = vr_ps_pool.tile([128, 1024], F32)
                for q in range(2):
                    nc.tensor.matmul(
                        vp[:, q * 512:(q + 1) * 512], r8[:],
                        v_sb[:, h * 1024 + q * 512:h * 1024 + (q + 1) * 512],
                        start=True, stop=True)
                if h == 0:
                    nc.scalar.activation(v_rep[:, :1024], vp[:], Act.Copy)
                else:
                    nc.vector.tensor_copy(v_rep[:, 1024:], vp[:])
            s_ps = s_ps_pool.tile([16, JM], F32)

            # per-tile state carried across the 1-tile software pipeline
            pend = [None]

            def issue_tail(t, expt, se):
                """rs/ct/cu + s-matmuls for tile t (issued one tile late)."""
                ut = u_tiles[t]
                rs = sm_pool.tile([128, 1], F32, tag="rs")
                nc.vector.reciprocal(rs[:], se[:])
                ct = sm_pool.tile([128, CH_J], BF16, tag="ct")
                nc.vector.tensor_scalar(ct[:], expt[:], rs[:], float(CH_J),
                                        Alu.mult, Alu.mult)
                cu = scr_pool.tile([128, JM], BF16, tag="cu")
                ctb = ct[:].rearrange("p (jp j2) -> p jp j2", j2=2) \
                    .unsqueeze(2).broadcast_to((128, JP, N_J, 2))
                nc.vector.tensor_mul(_pv(cu[:]), _pv(ut[:]), ctb)
                for q in range(4):
                    o0 = q * 512
                    nc.tensor.matmul(
                        s_ps[:, o0:o0 + 512], s16[:], cu[:, o0:o0 + 512],
                        start=(t == 0), stop=(t == N_T - 1))

            for t in range(N_T):
                ut = u_tiles[t]
                uv = scr_pool.tile([128, JM], BF16, tag="uv")
                nc.vector.tensor_mul(uv[:], ut[:], v_rep[:])
                uvv = _pv(uv[:])
                # m-reduce tree: 32 -> 16 -> 8 -> 4 -> 2 -> 1
                l1 = tr_pool.tile([128, JP * 16 * 2], BF16, tag="l1")
                nc.vector.tensor_add(
                    l1[:].rearrange("p (jp m j2) -> p jp m j2", m=16, j2=2),
                    uvv[:, :, 0:16, :], uvv[:, :, 16:32, :])
                l1v = l1[:].rearrange("p (jp m j2) -> p jp m j2", m=16, j2=2)
                l2 = tr_pool.tile([128, JP * 8 * 2], BF16, tag="l2")
                nc.vector.tensor_add(
                    l2[:].rearrange("p (jp m j2) -> p jp m j2", m=8, j2=2),
                    l1v[:, :, 0:8, :], l1v[:, :, 8:16, :])
                l2v = l2[:].rearrange("p (jp m j2) -> p jp m j2", m=8, j2=2)
                l3 = tr_pool.tile([128, JP * 4 * 2], F32, tag="l3")
                nc.vector.tensor_add(
                    l3[:].rearrange("p (jp m j2) -> p jp m j2", m=4, j2=2),
                    l2v[:, :, 0:4, :], l2v[:, :, 4:8, :])
                l3v = l3[:].rearrange("p (jp m j2) -> p jp m j2", m=4, j2=2)
                l4 = tr_pool.tile([128, JP * 2 * 2], F32, tag="l4")
                nc.vector.tensor_add(
                    l4[:].rearrange("p (jp m j2) -> p jp m j2", m=2, j2=2),
                    l3v[:, :, 0:2, :], l3v[:, :, 2:4, :])
                l4v = l4[:].rearrange("p (jp m j2) -> p jp m j2", m=2, j2=2)
                if r == 1:
                    a1 = a1_pool.tile([128, CH_J], F32)
                    a1_tiles.append(a1)
                    nc.vector.tensor_add(
                        a1[:].rearrange("p (jp m j2) -> p jp m j2",
                                        m=1, j2=2),
                        l4v[:, :, 0:1, :], l4v[:, :, 1:2, :])
                    logits = a1
                else:
                    a2 = sm_pool.tile([128, CH_J], F32, tag="a2")
                    nc.vector.tensor_add(
                        a2[:].rearrange("p (jp m j2) -> p jp m j2",
                                        m=1, j2=2),
                        l4v[:, :, 0:1, :], l4v[:, :, 1:2, :])
                    lg = sm_pool.tile([128, CH_J], F32, tag="lg")
                    nc.vector.tensor_add(lg[:], a2[:], a1_tiles[t][:])
                    logits = lg
                expt = sm_pool.tile([128, CH_J], BF16, tag="expt")
                se = sm_pool.tile([128, 1], F32, tag="se")
                nc.scalar.activation(expt[:], logits[:], Act.Exp,
                                     scale=8.0, accum_out=se[:])
                if pend[0] is not None:
                    issue_tail(*pend[0])
                pend[0] = (t, expt, se)
            issue_tail(*pend[0])

            s_sb2 = sv_pool.tile([16, JM], F32, tag="s_sb")
            nc.scalar.activation(s_sb2[:, :1024], s_ps[:, :1024], Act.Copy)
            nc.vector.tensor_copy(s_sb2[:, 1024:], s_ps[:, 1024:])
        v_sb = allreduce_squash(r, s_sb2)

    nc.sync.dma_start(v_d[:], v_sb[:])


def _host_inputs(inputs, w):
    """Build per-core input maps (host-side shard + block-diag pack).

    Free-dim layout for w/u/s/v on-device is paired: f = jp*64 + m*2 + j2
    with j = 2*jp + j2.
    """
    import ml_dtypes
    x = np.asarray(inputs, dtype=np.float32)
    w = np.asarray(w, dtype=np.float32)
    s16 = np.zeros((128, 16), dtype=np.float32)
    for i8 in range(8):
        for b in range(16):
            s16[i8 * 16 + b, b] = 1.0
    s16 = s16.astype(ml_dtypes.bfloat16)
    r8 = np.zeros((16, 128), dtype=np.float32)
    for i8 in range(8):
        for b in range(16):
            r8[b, i8 * 16 + b] = 1.0
    r8 = r8.astype(ml_dtypes.bfloat16)
    in_maps = []
    for k in range(N_CORES):
        i0 = k * I_LOC
        # (256, 16, 64, 32) -> (256*16, jp, j2, m) -> (.., jp, m, j2)
        wk = w[i0:i0 + I_LOC].reshape(I_LOC * N_I, JP, 2, N_J)
        wk = np.ascontiguousarray(wk.transpose(0, 1, 3, 2)).reshape(
            I_LOC * N_I, JM).astype(ml_dtypes.bfloat16)
        xk = x[:, i0:i0 + I_LOC, :]  # (B, 256, 16)
        xblk = np.zeros((N_T, 128, 128), dtype=np.float32)
        # xblk[t, i8*16+n, i8*16+b] = x[b, i0+8t+i8, n]
        xv = xk.transpose(1, 2, 0).reshape(N_T, 8, N_I, B)  # (t, i8, n, b)
        for i8 in range(8):
            xblk[:, i8 * 16:i8 * 16 + N_I, i8 * 16:i8 * 16 + B] = xv[:, i8]
        xblk = xblk.astype(ml_dtypes.bfloat16)
        in_maps.append({"xblk": xblk, "w": wk, "s16": s16, "r8": r8})
    return in_maps


def kernel(inputs, w, _trace=False):
    key = "nc"
    if key not in _CACHE:
        _CACHE[key] = _build_program()
    nc = _CACHE[key]
    in_maps = _host_inputs(inputs, w)
    res = run_bass_kernel_spmd(nc, in_maps, list(range(N_CORES)),
                               trace=_trace)
    vp = res.results[0]["v"].reshape(B, JP, N_J, 2)  # (b, jp, m, j2)
    v = np.ascontiguousarray(vp.transpose(0, 1, 3, 2)).reshape(
        B, CH_J, N_J).astype(np.float32)
    if _trace:
        kernel._last = res
    return v


# revision 5
# speedup vs baseline: 1.3633x; 1.0374x over previous
"""DenseCaps routing kernel for 8x Trainium2 NeuronCores.

Shapes (hardcoded): inputs (16, 2048, 16) f32, w (2048, 16, 64, 32) f32.
Sharding: ch_i (2048) split 8 ways -> 256 i's per core. Each core computes
u[b, i_loc, j, m] via a block-diagonal stationary matmul streaming its w
shard once (bf16), keeps u resident in SBUF (bf16), runs the 3-iteration
dynamic routing locally, and AllReduces the small s[b, j, m] partial sums.

Free-dim layout is "paired": f = jp*64 + m*2 + j2 with j = 2*jp + j2.
This makes the c-weighting of u a single 2x-mode tensor_tensor with a
stride-0 broadcast AP (no materialized broadcast), and the m-reduction a
tree of 2x-mode adds instead of a 1x tensor_reduce.

The s partials are AllReduced in two tile-halves per round so the first
collective overlaps the second half of the tile loop; a tiny warmup
AllReduce at kernel start absorbs the collective-stack init cost. The
softmax normalization (64/Z) is folded into the per-tile s-matmul
stationary, so the c-weighting multiply uses raw exp values.

Output v (16, 64, 32) f32 is identical on all cores; core 0's is returned.
"""

import sys
from contextlib import ExitStack

import numpy as np

sys.path.insert(0, "/opt/trn_rl_repo")

import concourse.bass as bass
import concourse.bacc as bacc
import concourse.tile as tile
from concourse import mybir
from concourse.bass_utils import run_bass_kernel_spmd

F32 = mybir.dt.float32
BF16 = mybir.dt.bfloat16

B = 16
CH_I = 2048
N_I = 16
CH_J = 64
N_J = 32
JM = CH_J * N_J  # 2048
JP = CH_J // 2  # 32 j-pairs
N_CORES = 8
I_LOC = CH_I // N_CORES  # 256
N_T = I_LOC // 8  # 32 production tiles, 8 i's each
H_T = N_T // 2
EPS = 1e-7

_CACHE = {}


def _build_program(trace=False):
    nc = bacc.Bacc("TRN2", target_bir_lowering=False, debug=False,
                   num_devices=N_CORES)

    xblk_d = nc.dram_tensor("xblk", [N_T, 128, 128], BF16, kind="ExternalInput")
    w_d = nc.dram_tensor("w", [I_LOC * N_I, JM], BF16, kind="ExternalInput")
    s16_d = nc.dram_tensor("s16", [128, 16], BF16, kind="ExternalInput")
    r8_d = nc.dram_tensor("r8", [16, 128], BF16, kind="ExternalInput")
    v_d = nc.dram_tensor("v", [B, JM], F32, kind="ExternalOutput")

    cc_in = [nc.dram_tensor(f"cc_in{h}", [B, JM], BF16) for h in range(6)]
    cc_out = [nc.dram_tensor(f"cc_out{h}", [B, JM], BF16, addr_space="Shared")
              for h in range(6)]
    ccw_in = nc.dram_tensor("ccw_in", [16, 16], F32)
    ccw_out = nc.dram_tensor("ccw_out", [16, 16], F32, addr_space="Shared")

    with tile.TileContext(nc) as tc, ExitStack() as ctx:
        _kernel_body(ctx, tc, xblk_d, w_d, s16_d, r8_d, v_d, cc_in, cc_out,
                     ccw_in, ccw_out)
    nc.compile()
    return nc


def _pv(ap):
    """View a [P, JM] AP as [P, jp, m, j2]."""
    return ap.rearrange("p (jp m j2) -> p jp m j2", m=N_J, j2=2)


def _kernel_body(ctx, tc, xblk_d, w_d, s16_d, r8_d, v_d, cc_in, cc_out,
                 ccw_in, ccw_out):
    nc = tc.nc
    Act = mybir.ActivationFunctionType
    Alu = mybir.AluOpType
    groups = [list(range(N_CORES))]

    const_pool = ctx.enter_context(tc.tile_pool(name="consts", bufs=1))
    s16 = const_pool.tile([128, 16], BF16)
    nc.sync.dma_start(s16[:], s16_d[:])
    r8 = const_pool.tile([16, 128], BF16)
    nc.sync.dma_start(r8[:], r8_d[:])
    eps_t = const_pool.tile([16, 1], F32)
    nc.vector.memset(eps_t[:], EPS)

    # warmup AllReduce: absorbs collective-stack init during phase A
    warm = const_pool.tile([16, 16], F32)
    nc.vector.memset(warm[:], 0.0)
    nc.sync.dma_start(ccw_in[:], warm[:])
    nc.gpsimd.collective_compute(
        "AllReduce", Alu.add, replica_groups=groups,
        ins=[ccw_in[:]], outs=[ccw_out[:]])

    # all xblk tiles resident up front (frees DMA for the w stream)
    xall = const_pool.tile([128, N_T * 128], BF16)
    for t in range(N_T):
        nc.sync.dma_start(xall[:, t * 128:(t + 1) * 128], xblk_d[t])

    u_pool = ctx.enter_context(tc.tile_pool(name="u", bufs=N_T))
    u_tiles = []

    # small persistent buffers
    sv_pool = ctx.enter_context(tc.tile_pool(name="sv", bufs=1))
    a1_pool = ctx.enter_context(tc.tile_pool(name="a1", bufs=N_T))
    a1_tiles = []

    def launch_ar(idx, s_ps, tag):
        """Copy s PSUM -> SBUF bf16, DMA to DRAM, AllReduce."""
        s_sb = sv_pool.tile([16, JM], BF16, tag=f"s_sb{tag}")
        nc.scalar.activation(s_sb[:, :1024], s_ps[:, :1024], Act.Copy)
        nc.vector.tensor_copy(s_sb[:, 1024:], s_ps[:, 1024:])
        nc.sync.dma_start(cc_in[idx][:], s_sb[:])
        nc.gpsimd.collective_compute(
            "AllReduce", Alu.add, replica_groups=groups,
            ins=[cc_in[idx][:]], outs=[cc_out[idx][:]])

    def merge_squash(r):
        """Fetch the two AllReduced halves, merge, squash -> v."""
        sfa = sv_pool.tile([16, JM], BF16, tag="sfa")
        nc.sync.dma_start(sfa[:], cc_out[2 * r][:])
        sfb = sv_pool.tile([16, JM], BF16, tag="sfb")
        nc.sync.dma_start(sfb[:], cc_out[2 * r + 1][:])
        sf = sv_pool.tile([16, JM], BF16, tag="sf")
        nc.vector.tensor_add(sf[:], sfa[:], sfb[:])
        # t2 = sf*sf, halves on Scalar (Square) and Vector
        t2 = sv_pool.tile([16, JM], BF16, tag="t2")
        nc.scalar.activation(t2[:, :1024], sf[:, :1024], Act.Square)
        nc.vector.tensor_mul(t2[:, 1024:], sf[:, 1024:], sf[:, 1024:])
        # tree-reduce over m: [16,(jp,m,j2)] -> [16,(jp,j2)]
        cur, mm = t2, N_J
        for lvl, dt in ((1, BF16), (2, BF16), (3, F32), (4, F32), (5, F32)):
            nxt = sv_pool.tile([16, JP * (mm // 2) * 2], dt, tag=f"q{lvl}")
            cv = cur[:].rearrange("p (jp m j2) -> p jp m j2", m=mm, j2=2)
            nc.vector.tensor_add(
                nxt[:].rearrange("p (jp m j2) -> p jp m j2", m=mm // 2, j2=2),
                cv[:, :, 0:mm // 2, :], cv[:, :, mm // 2:mm, :])
            cur, mm = nxt, mm // 2
        sq = cur  # [16, CH_J] f32
        rt = sv_pool.tile([16, CH_J], F32, tag="rt")
        nc.scalar.activation(rt[:], sq[:], Act.Sqrt, bias=eps_t[:])
        onep = sv_pool.tile([16, CH_J], F32, tag="onep")
        nc.vector.tensor_scalar_add(onep[:], sq[:], 1.0)
        den = sv_pool.tile([16, CH_J], F32, tag="den")
        nc.vector.tensor_mul(den[:], rt[:], onep[:])
        rec = sv_pool.tile([16, CH_J], F32, tag="rec")
        nc.vector.reciprocal(rec[:], den[:])
        fs = sv_pool.tile([16, CH_J], BF16, tag="fs")
        nc.vector.tensor_mul(fs[:], sq[:], rec[:])
        # v = sf * fs  (fs broadcast over m via stride-0 AP)
        fsb = fs[:].rearrange("p (jp j2) -> p jp j2", j2=2) \
            .unsqueeze(2).broadcast_to((16, JP, N_J, 2))
        vv = sv_pool.tile([16, JM], F32 if r == 2 else BF16, tag="vv")
        nc.vector.tensor_mul(_pv(vv[:]), _pv(sf[:]), fsb)
        return vv

    # ---------------- Phase A: produce u, accumulate s0 ----------------
    # Software-pipelined: s0 matmuls for tile t are issued during tile t+1.
    # s0 is accumulated in two tile-halves; the first half's AllReduce
    # overlaps the second half of the loop.
    with tc.tile_pool(name="wbuf", bufs=4) as w_pool, \
         tc.tile_pool(name="uprod", bufs=2, space="PSUM") as up_pool, \
         tc.tile_pool(name="s0ps", bufs=1, space="PSUM") as s0_pool:
        s0_ps = s0_pool.tile([16, JM], F32)

        def issue_s0(t):
            ut = u_tiles[t]
            for q in range(4):
                o0 = q * 512
                nc.tensor.matmul(
                    s0_ps[:, o0:o0 + 512], s16[:], ut[:, o0:o0 + 512],
                    start=(t % H_T == 0), stop=(t % H_T == H_T - 1))
            if t == H_T - 1:
                launch_ar(0, s0_ps, "a")

        for t in range(N_T):
            wt = w_pool.tile([128, JM], BF16)
            nc.sync.dma_start(wt[:], w_d[t * 128:(t + 1) * 128, :])
            xt = xall[:, t * 128:(t + 1) * 128]
            ut = u_pool.tile([128, JM], BF16)
            u_tiles.append(ut)
            ups = []
            for h in range(2):
                up = up_pool.tile([128, 1024], F32)
                ups.append(up)
                for q in range(2):
                    nc.tensor.matmul(
                        up[:, q * 512:(q + 1) * 512], xt,
                        wt[:, h * 1024 + q * 512: h * 1024 + (q + 1) * 512],
                        start=True, stop=True)
            # copies split across Scalar and Vector engines
            nc.scalar.activation(ut[:, :1024], ups[0][:], Act.Copy)
            nc.vector.tensor_copy(ut[:, 1024:], ups[1][:])
            if t > 0:
                issue_s0(t - 1)
        issue_s0(N_T - 1)
        launch_ar(1, s0_ps, "b")

    v_sb = merge_squash(0)

    # ---------------- Phases C: routing passes r=1,2 ----------------
    for r in (1, 2):
        with tc.tile_pool(name=f"vr{r}", bufs=1) as vr_pool, \
             tc.tile_pool(name=f"scr{r}", bufs=2) as scr_pool, \
             tc.tile_pool(name=f"tr{r}", bufs=2) as tr_pool, \
             tc.tile_pool(name=f"sm{r}", bufs=3) as sm_pool, \
             tc.tile_pool(name=f"vrps{r}", bufs=2, space="PSUM") as vr_ps_pool, \
             tc.tile_pool(name=f"sps{r}", bufs=1, space="PSUM") as s_ps_pool:
            # v_rep (128, JM) bf16: broadcast v over the 8 i-slots
            v_rep = vr_pool.tile([128, JM], BF16)
            for h in range(2):
                vp = vr_ps_pool.tile([128, 1024], F32)
                for q in range(2):
                    nc.tensor.matmul(
                        vp[:, q * 512:(q + 1) * 512], r8[:],
                        v_sb[:, h * 1024 + q * 512:h * 1024 + (q + 1) * 512],
                        start=True, stop=True)
                if h == 0:
                    nc.scalar.activation(v_rep[:, :1024], vp[:], Act.Copy)
                else:
                    nc.vector.tensor_copy(v_rep[:, 1024:], vp[:])
            s_ps = s_ps_pool.tile([16, JM], F32)

            pend = [None]

            def issue_tail(t, expt, se, r=r, s_ps=s_ps):
                """zs/cu + s-matmuls for tile t (issued one tile late)."""
                ut = u_tiles[t]
                rs = sm_pool.tile([128, 1], F32, tag="rs")
                nc.vector.reciprocal(rs[:], se[:])
                # fold 64/Z into the s-matmul stationary
                zs = sm_pool.tile([128, 16], BF16, tag="zs")
                nc.vector.tensor_scalar(zs[:], s16[:], rs[:], float(CH_J),
                                        Alu.mult, Alu.mult)
                cu = scr_pool.tile([128, JM], BF16, tag="cu")
                ctb = expt[:].rearrange("p (jp j2) -> p jp j2", j2=2) \
                    .unsqueeze(2).broadcast_to((128, JP, N_J, 2))
                nc.vector.tensor_mul(_pv(cu[:]), _pv(ut[:]), ctb)
                for q in range(4):
                    o0 = q * 512
                    nc.tensor.matmul(
                        s_ps[:, o0:o0 + 512], zs[:], cu[:, o0:o0 + 512],
                        start=(t % H_T == 0), stop=(t % H_T == H_T - 1))
                if t == H_T - 1:
                    launch_ar(2 * r, s_ps, "a")

            for t in range(N_T):
                ut = u_tiles[t]
                uv = scr_pool.tile([128, JM], BF16, tag="uv")
                nc.vector.tensor_mul(uv[:], ut[:], v_rep[:])
                # m-reduce tree: 32 -> 16 -> 8 -> 4 -> 2 -> 1 (all bf16)
                cur, mm = uv, N_J
                for lvl in range(4):
                    nxt = tr_pool.tile([128, JP * (mm // 2) * 2], BF16,
                                       tag=f"l{lvl}")
                    cv = cur[:].rearrange("p (jp m j2) -> p jp m j2",
                                          m=mm, j2=2)
                    nc.vector.tensor_add(
                        nxt[:].rearrange("p (jp m j2) -> p jp m j2",
                                         m=mm // 2, j2=2),
                        cv[:, :, 0:mm // 2, :], cv[:, :, mm // 2:mm, :])
                    cur, mm = nxt, mm // 2
                l4v = cur[:].rearrange("p (jp m j2) -> p jp m j2", m=2, j2=2)
                if r == 1:
                    a1 = a1_pool.tile([128, CH_J], BF16)
                    a1_tiles.append(a1)
                    nc.vector.tensor_add(
                        a1[:].rearrange("p (jp m j2) -> p jp m j2",
                                        m=1, j2=2),
                        l4v[:, :, 0:1, :], l4v[:, :, 1:2, :])
                    logits = a1
                else:
                    a2 = sm_pool.tile([128, CH_J], BF16, tag="a2")
                    nc.vector.tensor_add(
                        a2[:].rearrange("p (jp m j2) -> p jp m j2",
                                        m=1, j2=2),
                        l4v[:, :, 0:1, :], l4v[:, :, 1:2, :])
                    lg = sm_pool.tile([128, CH_J], BF16, tag="lg")
                    nc.vector.tensor_add(lg[:], a2[:], a1_tiles[t][:])
                    logits = lg
                expt = sm_pool.tile([128, CH_J], BF16, tag="expt")
                se = sm_pool.tile([128, 1], F32, tag="se")
                nc.scalar.activation(expt[:], logits[:], Act.Exp,
                                     scale=8.0, accum_out=se[:])
                if pend[0] is not None:
                    issue_tail(*pend[0])
                pend[0] = (t, expt, se)
            issue_tail(*pend[0])
            launch_ar(2 * r + 1, s_ps, "b")
        v_sb = merge_squash(r)

    nc.sync.dma_start(v_d[:], v_sb[:])


def _host_inputs(inputs, w):
    """Build per-core input maps (host-side shard + block-diag pack).

    Free-dim layout for w/u/s/v on-device is paired: f = jp*64 + m*2 + j2
    with j = 2*jp + j2.
    """
    import ml_dtypes
    x = np.asarray(inputs, dtype=np.float32)
    w = np.asarray(w, dtype=np.float32)
    s16 = np.zeros((128, 16), dtype=np.float32)
    for i8 in range(8):
        for b in range(16):
            s16[i8 * 16 + b, b] = 1.0
    s16 = s16.astype(ml_dtypes.bfloat16)
    r8 = np.zeros((16, 128), dtype=np.float32)
    for i8 in range(8):
        for b in range(16):
            r8[b, i8 * 16 + b] = 1.0
    r8 = r8.astype(ml_dtypes.bfloat16)
    in_maps = []
    for k in range(N_CORES):
        i0 = k * I_LOC
        # (256, 16, 64, 32) -> (256*16, jp, j2, m) -> (.., jp, m, j2)
        wk = w[i0:i0 + I_LOC].reshape(I_LOC * N_I, JP, 2, N_J)
        wk = np.ascontiguousarray(wk.transpose(0, 1, 3, 2)).reshape(
            I_LOC * N_I, JM).astype(ml_dtypes.bfloat16)
        xk = x[:, i0:i0 + I_LOC, :]  # (B, 256, 16)
        xblk = np.zeros((N_T, 128, 128), dtype=np.float32)
        # xblk[t, i8*16+n, i8*16+b] = x[b, i0+8t+i8, n]
        xv = xk.transpose(1, 2, 0).reshape(N_T, 8, N_I, B)  # (t, i8, n, b)
        for i8 in range(8):
            xblk[:, i8 * 16:i8 * 16 + N_I, i8 * 16:i8 * 16 + B] = xv[:, i8]
        xblk = xblk.astype(ml_dtypes.bfloat16)
        in_maps.append({"xblk": xblk, "w": wk, "s16": s16, "r8": r8})
    return in_maps


def kernel(inputs, w, _trace=False):
    key = "nc"
    if key not in _CACHE:
        _CACHE[key] = _build_program()
    nc = _CACHE[key]
    in_maps = _host_inputs(inputs, w)
    res = run_bass_kernel_spmd(nc, in_maps, list(range(N_CORES)),
                               trace=_trace)
    vp = res.results[0]["v"].reshape(B, JP, N_J, 2)  # (b, jp, m, j2)
    v = np.ascontiguousarray(vp.transpose(0, 1, 3, 2)).reshape(
        B, CH_J, N_J).astype(np.float32)
    if _trace:
        kernel._last = res
    return v
